# revision 24
# baseline (speedup 1.0000x reference)
"""Trainium2 kernel for nn_LAM_Module_19052474925494 (mixed fp8/fp16 matmul).

Reference computation (B,N,C,H,W = 16,10,128,48,48):
  q = k = x.reshape(B,N,D), D = C*H*W = 294912
  s0 = (1-pd)*k[n] + pd*k[n+1]        (indices mod N)
  s1 = ld*((1-pd)*k[n+1] + pd*k[n+2])
  logits = [q.s0, q.s1]; attn = softmax(logits); out = attn0*s0 + attn1*s1
  feat = out.reshape(B, N*C, H, W)
  result = conv1x1(conv_w, feat) + conv_b + x.reshape(B, N*C, H, W)

For this input distribution the 2-way softmax saturates exactly to [1, 0]
(logit gap ~1.5e5; guarded host-side), so feat is linear in x and folds into
the conv weights: result[b] = W_eff @ X_b + bias + X_b.  The residual X_b is
added on the HOST (not folded into W_eff): device output is just W_eff @ X_b.

Device kernel: per-core [1280 x 1280] @ [1280 x 2304] matmul for 2 batch
items (data-parallel over batch, 8 cores). Mixed precision along the
contraction dim to exploit the PE's fp8 DoubleRow mode (2 rows/cycle, 2x
fp16 rate): FP8_CHUNKS of the ten 128-channel K-chunks run as e4m3
DoubleRow pairs, the rest as fp16 at 1x. W is scaled by SW=64 and X by SX=8
host-side so e4m3 sees a healthy binade range; the PSUM result is descaled
by 1/(SW*SX) in the bias-add vector op. Simulated end-to-end rel-err
(FP8_CHUNKS=6): 1.72e-2 vs the 2e-2 gate; pure fp16 floor is 192us/core,
this config's PE floor is ~134us/core.
"""

import numpy as np

B, N, C, H, W = 16, 10, 128, 48, 48
NCh = N * C   # 1280 channels
HW = H * W    # 2304 spatial
NCORES = 8
BB = B // NCORES  # batch items per core

# Tunables (test.py may override before first kernel() call)
FP8_CHUNKS = 6        # even; this many 128-ch K-chunks run as e4m3 DoubleRow
SW = 64.0             # host-side weight scale before fp8/fp16 quantization
SX = 8.0              # host-side activation scale
OUT_F16 = True        # device writes f16 output (halves write traffic)
WARMUP_MMS = 16
OUT_BUFS = 16
PS_BUFS = 8
COLT = [(0, 512), (512, 512), (1024, 512), (1536, 512), (2048, 256)]
NT = len(COLT)
TRACE = False
TRACE_CORES = None
LAST_RESULT = None

NP8 = FP8_CHUNKS // 2
N16 = N - FP8_CHUNKS

_cache = {}


def _build_nc():
    import concourse.bacc as bacc
    import concourse.mybir as mybir
    from concourse.tile import TileContext

    global NP8, N16
    NP8 = FP8_CHUNKS // 2
    N16 = N - FP8_CHUNKS

    f32 = mybir.dt.float32
    f16 = mybir.dt.float16
    f8 = mybir.dt.float8e4
    DR = mybir.MatmulPerfMode.DoubleRow
    out_dt = f16 if OUT_F16 else f32

    nc = bacc.Bacc(None, target_bir_lowering=False, debug=False)
    xs8 = nc.dram_tensor("xs8", [BB, NP8, C, 2 * HW], f8, kind="ExternalInput")
    xs16 = nc.dram_tensor("xs16", [BB, max(N16, 1), C, HW], f16, kind="ExternalInput")
    wt8 = nc.dram_tensor("wt8", [NP8, C, 2 * NCh], f8, kind="ExternalInput")
    wt16 = nc.dram_tensor("wt16", [max(N16, 1), C, NCh], f16, kind="ExternalInput")
    bias = nc.dram_tensor("bias", [C, N], f32, kind="ExternalInput")
    # Output is written as per-group contiguous [C, 512] slabs (ob-major,
    # coltile-minor) so each drain DMA is a single sequential HBM burst; the
    # host reassembles. Last coltile uses only 256 of its 512 slot columns.
    out = nc.dram_tensor("out", [BB, N, NT, C, 512], out_dt, kind="ExternalOutput")

    descale = 1.0 / (SW * SX)

    with TileContext(nc) as tc:
        with (
            tc.tile_pool(name="wtp", bufs=1) as wt_pool,
            tc.tile_pool(name="biasp", bufs=1) as bias_pool,
            tc.tile_pool(name="xp", bufs=1) as x_pool,
            tc.tile_pool(name="psp", bufs=PS_BUFS, space="PSUM") as psum_pool,
            tc.tile_pool(name="op", bufs=OUT_BUFS) as out_pool,
        ):
            bias_sb = bias_pool.tile([C, N], f32, name="bias_sb")

            if WARMUP_MMS:
                # PE warm-up: zero-dependency DR matmuls keep the PE busy and
                # ramp the p-state while preambles finish and DMA streams in.
                wsc = bias_pool.tile([C, 2, 256], f8, name="warm_sc")
                nc.gpsimd.memset(wsc[:], 0.0)
                wps = psum_pool.tile([C, 512], f32, tag="ps", name="warm_ps")
                for wi in range(WARMUP_MMS):
                    nc.tensor.matmul(
                        wps[:, :256], wsc[:, :, :C], wsc[:],
                        start=True, stop=True, perf_mode=DR,
                    )

            wt8_sb, wt16_sb, x8_sb, x16_sb = {}, {}, {}, {}

            def load_wt(j):
                # Weight loads are split across the scalar and gpsimd queues
                # so chunk arrival order matches the chase sweep's consumption
                # order (scalar: p0, p1, k2, k3; gpsimd: p2, k0, k1).
                if j < NP8:
                    t = wt_pool.tile(
                        [C, 2, NCh], f8, tag=f"wt8_{j}", name=f"wt8_sb{j}"
                    )
                    eng = nc.gpsimd if j == NP8 - 1 and NP8 > 2 else nc.scalar
                    eng.dma_start(
                        out=t[:],
                        in_=wt8[j].rearrange("c (i o) -> c i o", i=2),
                    )
                    wt8_sb[j] = t
                else:
                    k = j - NP8
                    t = wt_pool.tile([C, NCh], f16, tag=f"wt16_{k}", name=f"wt16_sb{k}")
                    eng = nc.gpsimd if k < N16 // 2 else nc.scalar
                    eng.dma_start(out=t[:], in_=wt16[k, :, :])
                    wt16_sb[k] = t

            def alloc_x(it):
                for p in range(NP8):
                    x8_sb[(it, p)] = x_pool.tile(
                        [C, 2, HW], f8, tag=f"x8_{it}_{p}", name=f"x8_{it}_{p}"
                    )
                for k in range(N16):
                    x16_sb[(it, k)] = x_pool.tile(
                        [C, HW], f16, tag=f"x16_{it}_{k}", name=f"x16_{it}_{k}"
                    )

            def load_stripe(it, t_idx, chunks=None, f16_eng=None):
                c0, cw = COLT[t_idx]
                for j in chunks if chunks is not None else range(NP8 + N16):
                    if j < NP8:
                        t = x8_sb[(it, j)]
                        nc.sync.dma_start(
                            out=t[:, :, c0 : c0 + cw],
                            in_=xs8[it, j].rearrange("c (i n) -> c i n", i=2)[
                                :, :, c0 : c0 + cw
                            ],
                        )
                    else:
                        k = j - NP8
                        (f16_eng or nc.sync).dma_start(
                            out=x16_sb[(it, k)][:, c0 : c0 + cw],
                            in_=xs16[it, k, :, c0 : c0 + cw],
                        )

            def mm(ps, it, j, ob, c0, cw, start, stop):
                if j < NP8:
                    nc.tensor.matmul(
                        ps[:, :cw],
                        wt8_sb[j][:, :, ob * C : (ob + 1) * C],
                        x8_sb[(it, j)][:, :, c0 : c0 + cw],
                        start=start, stop=stop, perf_mode=DR,
                    )
                else:
                    k = j - NP8
                    nc.tensor.matmul(
                        ps[:, :cw],
                        wt16_sb[k][:, ob * C : (ob + 1) * C],
                        x16_sb[(it, k)][:, c0 : c0 + cw],
                        start=start, stop=stop,
                    )

            nj = NP8 + N16
            tix = {c0: t for t, (c0, _) in enumerate(COLT)}
            drain_ct = [0]

            def drain(ps, it, ob, c0, cw):
                osb = out_pool.tile([C, 512], out_dt, tag="o", name=f"o_{it}_{ob}_{c0}")
                nc.vector.tensor_scalar(
                    osb[:, :cw], ps[:, :cw], descale, bias_sb[:, ob : ob + 1],
                    mybir.AluOpType.mult, mybir.AluOpType.add,
                )
                # Rotate output DMAs across queues so the write stream keeps
                # up with compute; sync joins once item1's X loads are done.
                engs = (nc.gpsimd, nc.scalar) if it == 0 else (
                    nc.gpsimd, nc.scalar, nc.sync
                )
                eng = engs[drain_ct[0] % len(engs)]
                drain_ct[0] += 1
                eng.dma_start(out=out[it, ob, tix[c0], :, :cw], in_=osb[:, :cw])

            def group(it, ob, c0, cw):
                ps = psum_pool.tile([C, 512], f32, tag="ps", name=f"ps_{it}_{ob}_{c0}")
                for j in range(nj):
                    mm(ps, it, j, ob, c0, cw, j == 0, j == nj - 1)
                drain(ps, it, ob, c0, cw)

            alloc_x(0)
            alloc_x(1)

            # Weights + item0 stripe0, interleaved chunk-wise so the first
            # kb-outer sweep can chase arrivals.
            for j in range(nj):
                load_wt(j)
                load_stripe(0, 0, [j])

            # bias is only needed by the first drain; issue it after the
            # startup-critical loads.
            nc.sync.dma_start(out=bias_sb[:], in_=bias[:])
            load_stripe(0, 1)

            # Chase sweeps: item0 coltile0 then coltile1, kb-outer across 8
            # output blocks at once -> each arriving (wt, x) chunk unlocks 8
            # matmuls, and straggling chunk DMAs only stall their own chunk.
            for t in (0, 1):
                c0, cw = COLT[t]
                pst = {
                    ob: psum_pool.tile([C, 512], f32, tag="ps", name=f"psw{t}_{ob}")
                    for ob in range(8)
                }
                for j in range(nj):
                    for ob in range(8):
                        mm(pst[ob], 0, j, ob, c0, cw, j == 0, j == nj - 1)
                for ob in range(8):
                    drain(pst[ob], 0, ob, c0, cw)
            for t in (0, 1):
                c0, cw = COLT[t]
                for ob in (8, 9):
                    group(0, ob, c0, cw)

            # item0 coltiles 2..4, with item0/item1 stripe loads interleaved
            # ahead of need.
            preload = [(0, 2), (1, 0), (0, 3), (1, 1), (0, 4), (1, 2), (1, 3), (1, 4)]
            load_stripe(*preload[0])
            load_stripe(*preload[1])
            pi = 2
            for t in range(2, len(COLT)):
                for _ in range(2):
                    if pi < len(preload):
                        load_stripe(*preload[pi])
                        pi += 1
                c0, cw = COLT[t]
                for ob in range(N):
                    group(0, ob, c0, cw)
            while pi < len(preload):
                load_stripe(*preload[pi])
                pi += 1
            for t in range(len(COLT)):
                c0, cw = COLT[t]
                for ob in range(N):
                    group(1, ob, c0, cw)
    nc.finalize()
    return nc


def kernel(x, pos_dec, length_dec, conv_w, conv_b):
    global LAST_RESULT
    import ml_dtypes
    from concourse.bass_utils import run_bass_kernel_spmd

    e4 = ml_dtypes.float8_e4m3
    pd = np.asarray(pos_dec, dtype=np.float32)
    ld = np.asarray(length_dec, dtype=np.float32)
    Wm = np.asarray(conv_w, dtype=np.float32)
    x = np.asarray(x, dtype=np.float32).reshape(B, N, C * H * W)

    # Guard: verify the 2-way softmax saturates to [1, 0] for this input.
    g0 = np.einsum("bnd,bnd->bn", x, x)
    x1 = np.roll(x, -1, axis=1)
    g1 = np.einsum("bnd,bnd->bn", x, x1)
    g2 = np.einsum("bnd,bnd->bn", x, np.roll(x, -2, axis=1))
    l0 = (1.0 - pd) * g0 + pd * g1
    l1 = ld * ((1.0 - pd) * g1 + pd * g2)
    saturated = bool((l0 - l1).min() > 25.0)

    if saturated:
        # attn == [1, 0] exactly in fp32 -> feat_n = (1-pd_n) x_n + pd_n x_{n+1};
        # fold the interpolation into the weights (residual stays on host).
        W_eff = np.empty_like(Wm)
        for m in range(N):
            pm = (m - 1) % N
            W_eff[:, m * C : (m + 1) * C] = \
                (1.0 - pd[m]) * Wm[:, m * C : (m + 1) * C] + \
                pd[pm] * Wm[:, pm * C : (pm + 1) * C]
        feed = x
    else:
        # General path: materialize feat with the true attention weights.
        gap = l1 - l0
        a1 = 1.0 / (1.0 + np.exp(np.clip(-gap, -87.0, 87.0)))
        a0 = 1.0 - a1
        c0 = (a0 * (1.0 - pd))[:, :, None]
        c1 = (a0 * pd + a1 * ld * (1.0 - pd))[:, :, None]
        c2 = (a1 * ld * pd)[:, :, None]
        feed = c0 * x + c1 * x1 + c2 * np.roll(x, -2, axis=1)
        W_eff = Wm

    nsplit = FP8_CHUNKS * C
    feed = feed.reshape(B, NCh, HW)
    # X8: [B, NP8, C, 2*HW], partition row c holds channels (256p+c, 256p+128+c)
    f8part = (feed[:, :nsplit, :] * SX).reshape(B, NP8, 2, C, HW)
    X8 = np.ascontiguousarray(f8part.transpose(0, 1, 3, 2, 4)).reshape(
        B, NP8, C, 2 * HW
    )
    X8 = np.clip(X8, -240, 240).astype(e4)
    X16 = np.ascontiguousarray(
        (feed[:, nsplit:, :] * SX).reshape(B, max(N16, 1), C, HW)
    ).astype(np.float16)

    WT = (W_eff * SW).T  # [c_in, o]
    W8 = WT[:nsplit].reshape(NP8, 2, C, NCh).transpose(0, 2, 1, 3)
    W8 = np.clip(np.ascontiguousarray(W8).reshape(NP8, C, 2 * NCh), -240, 240).astype(e4)
    W16 = np.ascontiguousarray(WT[nsplit:].reshape(max(N16, 1), C, NCh)).astype(
        np.float16
    )
    bias_t = np.ascontiguousarray(
        np.asarray(conv_b, dtype=np.float32).reshape(N, C).T
    )  # [C, N]: column ob = biases of output block ob

    if "nc" not in _cache:
        _cache["nc"] = _build_nc()
    nc = _cache["nc"]

    in_maps = [
        {
            "xs8": X8[c * BB : (c + 1) * BB],
            "xs16": X16[c * BB : (c + 1) * BB],
            "wt8": W8,
            "wt16": W16,
            "bias": bias_t,
        }
        for c in range(NCORES)
    ]
    res = None
    for attempt in range(3):
        try:
            res = run_bass_kernel_spmd(
                nc, in_maps, core_ids=list(range(NCORES)), trace=TRACE,
                trace_cores=TRACE_CORES,
            )
            break
        except Exception:
            # The PJRT/axon dispatch occasionally hits a transient
            # device-unrecoverable error; a retry re-initializes and succeeds.
            if attempt == 2:
                raise
            import time

            time.sleep(2.0)
    LAST_RESULT = res
    slabs = np.concatenate(
        [np.asarray(res.results[c]["out"]) for c in range(NCORES)], axis=0
    )  # [B, N, NT, C, 512]
    out = np.empty((B, NCh, HW), np.float32)
    for t, (c0, cw) in enumerate(COLT):
        out[:, :, c0 : c0 + cw] = (
            slabs[:, :, t, :, :cw].reshape(B, NCh, cw).astype(np.float32)
        )
    out = out + feed if saturated else out + x.reshape(B, NCh, HW)
    # residual is x in both paths; in the saturated path feed IS x.
    return out.reshape(B, NCh, H, W)


# revision 25
# speedup vs baseline: 1.0243x; 1.0243x over previous
"""Trainium2 kernel for nn_LAM_Module_19052474925494 (mixed fp8/fp16 matmul).

Reference computation (B,N,C,H,W = 16,10,128,48,48):
  q = k = x.reshape(B,N,D), D = C*H*W = 294912
  s0 = (1-pd)*k[n] + pd*k[n+1]        (indices mod N)
  s1 = ld*((1-pd)*k[n+1] + pd*k[n+2])
  logits = [q.s0, q.s1]; attn = softmax(logits); out = attn0*s0 + attn1*s1
  feat = out.reshape(B, N*C, H, W)
  result = conv1x1(conv_w, feat) + conv_b + x.reshape(B, N*C, H, W)

For this input distribution the 2-way softmax saturates exactly to [1, 0]
(logit gap ~1.5e5; guarded host-side), so feat is linear in x and folds into
the conv weights: result[b] = W_eff @ X_b + bias + X_b.  The residual X_b is
added on the HOST (not folded into W_eff): device output is just W_eff @ X_b.

Device kernel: per-core [1280 x 1280] @ [1280 x 2304] matmul for 2 batch
items (data-parallel over batch, 8 cores). Mixed precision along the
contraction dim to exploit the PE's fp8 DoubleRow mode (2 rows/cycle, 2x
fp16 rate): FP8_CHUNKS of the ten 128-channel K-chunks run as e4m3
DoubleRow pairs, the rest as fp16 at 1x. W is scaled by SW=64 and X by SX=8
host-side so e4m3 sees a healthy binade range; the PSUM result is descaled
by 1/(SW*SX) in the bias-add vector op. Simulated end-to-end rel-err
(FP8_CHUNKS=6): 1.72e-2 vs the 2e-2 gate; pure fp16 floor is 192us/core,
this config's PE floor is ~134us/core.
"""

import numpy as np

B, N, C, H, W = 16, 10, 128, 48, 48
NCh = N * C   # 1280 channels
HW = H * W    # 2304 spatial
NCORES = 8
BB = B // NCORES  # batch items per core

# Tunables (test.py may override before first kernel() call)
FP8_CHUNKS = 6        # even; this many 128-ch K-chunks run as e4m3 DoubleRow
SW = 64.0             # host-side weight scale before fp8/fp16 quantization
SX = 8.0              # host-side activation scale
OUT_F16 = True        # device writes f16 output (halves write traffic)
WARMUP_MMS = 16
OUT_BUFS = 16
PS_BUFS = 8
COLT = [(0, 512), (512, 512), (1024, 512), (1536, 512), (2048, 256)]
NT = len(COLT)
TRACE = False
TRACE_CORES = None
LAST_RESULT = None

NP8 = FP8_CHUNKS // 2
N16 = N - FP8_CHUNKS

_cache = {}


def _build_nc():
    import concourse.bacc as bacc
    import concourse.mybir as mybir
    from concourse.tile import TileContext

    global NP8, N16
    NP8 = FP8_CHUNKS // 2
    N16 = N - FP8_CHUNKS

    f32 = mybir.dt.float32
    f16 = mybir.dt.float16
    f8 = mybir.dt.float8e4
    DR = mybir.MatmulPerfMode.DoubleRow
    out_dt = f16 if OUT_F16 else f32

    nc = bacc.Bacc(None, target_bir_lowering=False, debug=False)
    xs8 = nc.dram_tensor("xs8", [BB, NP8, C, 2 * HW], f8, kind="ExternalInput")
    xs16 = nc.dram_tensor("xs16", [BB, max(N16, 1), C, HW], f16, kind="ExternalInput")
    wt8 = nc.dram_tensor("wt8", [NP8, C, 2 * NCh], f8, kind="ExternalInput")
    wt16 = nc.dram_tensor("wt16", [max(N16, 1), C, NCh], f16, kind="ExternalInput")
    bias = nc.dram_tensor("bias", [C, N], f32, kind="ExternalInput")
    # Output is written as per-group contiguous [C, 512] slabs (ob-major,
    # coltile-minor) so each drain DMA is a single sequential HBM burst; the
    # host reassembles. Last coltile uses only 256 of its 512 slot columns.
    out = nc.dram_tensor("out", [BB, N, NT, C, 512], out_dt, kind="ExternalOutput")

    descale = 1.0 / (SW * SX)

    with TileContext(nc) as tc:
        with (
            tc.tile_pool(name="wtp", bufs=1) as wt_pool,
            tc.tile_pool(name="biasp", bufs=1) as bias_pool,
            tc.tile_pool(name="xp", bufs=1) as x_pool,
            tc.tile_pool(name="psp", bufs=PS_BUFS, space="PSUM") as psum_pool,
            tc.tile_pool(name="op", bufs=OUT_BUFS) as out_pool,
        ):
            bias_sb = bias_pool.tile([C, N], f32, name="bias_sb")

            if WARMUP_MMS:
                # PE warm-up: zero-dependency DR matmuls keep the PE busy and
                # ramp the p-state while preambles finish and DMA streams in.
                wsc = bias_pool.tile([C, 2, 256], f8, name="warm_sc")
                nc.gpsimd.memset(wsc[:], 0.0)
                wps = psum_pool.tile([C, 512], f32, tag="ps", name="warm_ps")
                for wi in range(WARMUP_MMS):
                    nc.tensor.matmul(
                        wps[:, :256], wsc[:, :, :C], wsc[:],
                        start=True, stop=True, perf_mode=DR,
                    )

            wt8_sb, wt16_sb, x8_sb, x16_sb = {}, {}, {}, {}

            def load_wt(j):
                # Weight loads are split across the scalar and gpsimd queues
                # so chunk arrival order matches the chase sweep's consumption
                # order (scalar: p0, p1, k2, k3; gpsimd: p2, k0, k1).
                if j < NP8:
                    t = wt_pool.tile(
                        [C, 2, NCh], f8, tag=f"wt8_{j}", name=f"wt8_sb{j}"
                    )
                    eng = nc.gpsimd if j == NP8 - 1 and NP8 > 2 else nc.scalar
                    eng.dma_start(
                        out=t[:],
                        in_=wt8[j].rearrange("c (i o) -> c i o", i=2),
                    )
                    wt8_sb[j] = t
                else:
                    k = j - NP8
                    t = wt_pool.tile([C, NCh], f16, tag=f"wt16_{k}", name=f"wt16_sb{k}")
                    eng = nc.gpsimd if k < N16 // 2 else nc.scalar
                    eng.dma_start(out=t[:], in_=wt16[k, :, :])
                    wt16_sb[k] = t

            def alloc_x(it):
                for p in range(NP8):
                    x8_sb[(it, p)] = x_pool.tile(
                        [C, 2, HW], f8, tag=f"x8_{it}_{p}", name=f"x8_{it}_{p}"
                    )
                for k in range(N16):
                    x16_sb[(it, k)] = x_pool.tile(
                        [C, HW], f16, tag=f"x16_{it}_{k}", name=f"x16_{it}_{k}"
                    )

            def load_stripe(it, t_idx, chunks=None, f16_eng=None):
                c0, cw = COLT[t_idx]
                for j in chunks if chunks is not None else range(NP8 + N16):
                    if j < NP8:
                        t = x8_sb[(it, j)]
                        nc.sync.dma_start(
                            out=t[:, :, c0 : c0 + cw],
                            in_=xs8[it, j].rearrange("c (i n) -> c i n", i=2)[
                                :, :, c0 : c0 + cw
                            ],
                        )
                    else:
                        k = j - NP8
                        (f16_eng or nc.sync).dma_start(
                            out=x16_sb[(it, k)][:, c0 : c0 + cw],
                            in_=xs16[it, k, :, c0 : c0 + cw],
                        )

            def mm(ps, it, j, ob, c0, cw, start, stop):
                if j < NP8:
                    nc.tensor.matmul(
                        ps[:, :cw],
                        wt8_sb[j][:, :, ob * C : (ob + 1) * C],
                        x8_sb[(it, j)][:, :, c0 : c0 + cw],
                        start=start, stop=stop, perf_mode=DR,
                    )
                else:
                    k = j - NP8
                    nc.tensor.matmul(
                        ps[:, :cw],
                        wt16_sb[k][:, ob * C : (ob + 1) * C],
                        x16_sb[(it, k)][:, c0 : c0 + cw],
                        start=start, stop=stop,
                    )

            nj = NP8 + N16
            tix = {c0: t for t, (c0, _) in enumerate(COLT)}
            drain_ct = [0]

            def drain(ps, it, ob, c0, cw):
                osb = out_pool.tile([C, 512], out_dt, tag="o", name=f"o_{it}_{ob}_{c0}")
                nc.vector.tensor_scalar(
                    osb[:, :cw], ps[:, :cw], descale, bias_sb[:, ob : ob + 1],
                    mybir.AluOpType.mult, mybir.AluOpType.add,
                )
                # Alternate output DMAs across two queues so the write stream
                # keeps up with compute.
                eng = nc.gpsimd if drain_ct[0] % 2 == 0 else nc.scalar
                drain_ct[0] += 1
                eng.dma_start(out=out[it, ob, tix[c0], :, :cw], in_=osb[:, :cw])

            def group(it, ob, c0, cw):
                ps = psum_pool.tile([C, 512], f32, tag="ps", name=f"ps_{it}_{ob}_{c0}")
                for j in range(nj):
                    mm(ps, it, j, ob, c0, cw, j == 0, j == nj - 1)
                drain(ps, it, ob, c0, cw)

            alloc_x(0)
            alloc_x(1)

            # Weights + item0 stripe0, interleaved chunk-wise so the first
            # kb-outer sweep can chase arrivals.
            for j in range(nj):
                load_wt(j)
                load_stripe(0, 0, [j])

            # bias is only needed by the first drain; issue it after the
            # startup-critical loads.
            nc.sync.dma_start(out=bias_sb[:], in_=bias[:])
            load_stripe(0, 1)

            # Chase sweeps: item0 coltile0 then coltile1, kb-outer across 8
            # output blocks at once -> each arriving (wt, x) chunk unlocks 8
            # matmuls, and straggling chunk DMAs only stall their own chunk.
            for t in (0, 1):
                c0, cw = COLT[t]
                pst = {
                    ob: psum_pool.tile([C, 512], f32, tag="ps", name=f"psw{t}_{ob}")
                    for ob in range(8)
                }
                for j in range(nj):
                    for ob in range(8):
                        mm(pst[ob], 0, j, ob, c0, cw, j == 0, j == nj - 1)
                for ob in range(8):
                    drain(pst[ob], 0, ob, c0, cw)
            for t in (0, 1):
                c0, cw = COLT[t]
                for ob in (8, 9):
                    group(0, ob, c0, cw)

            # item0 coltiles 2..4, with item0/item1 stripe loads interleaved
            # ahead of need.
            preload = [(0, 2), (1, 0), (0, 3), (1, 1), (0, 4), (1, 2), (1, 3), (1, 4)]
            load_stripe(*preload[0])
            load_stripe(*preload[1])
            pi = 2
            for t in range(2, len(COLT)):
                for _ in range(2):
                    if pi < len(preload):
                        load_stripe(*preload[pi])
                        pi += 1
                c0, cw = COLT[t]
                for ob in range(N):
                    group(0, ob, c0, cw)
            while pi < len(preload):
                load_stripe(*preload[pi])
                pi += 1
            for t in range(len(COLT)):
                c0, cw = COLT[t]
                for ob in range(N):
                    group(1, ob, c0, cw)
    nc.finalize()
    return nc


def kernel(x, pos_dec, length_dec, conv_w, conv_b):
    global LAST_RESULT
    import ml_dtypes
    from concourse.bass_utils import run_bass_kernel_spmd

    e4 = ml_dtypes.float8_e4m3
    pd = np.asarray(pos_dec, dtype=np.float32)
    ld = np.asarray(length_dec, dtype=np.float32)
    Wm = np.asarray(conv_w, dtype=np.float32)
    x = np.asarray(x, dtype=np.float32).reshape(B, N, C * H * W)

    # Guard: verify the 2-way softmax saturates to [1, 0] for this input.
    g0 = np.einsum("bnd,bnd->bn", x, x)
    x1 = np.roll(x, -1, axis=1)
    g1 = np.einsum("bnd,bnd->bn", x, x1)
    g2 = np.einsum("bnd,bnd->bn", x, np.roll(x, -2, axis=1))
    l0 = (1.0 - pd) * g0 + pd * g1
    l1 = ld * ((1.0 - pd) * g1 + pd * g2)
    saturated = bool((l0 - l1).min() > 25.0)

    if saturated:
        # attn == [1, 0] exactly in fp32 -> feat_n = (1-pd_n) x_n + pd_n x_{n+1};
        # fold the interpolation into the weights (residual stays on host).
        W_eff = np.empty_like(Wm)
        for m in range(N):
            pm = (m - 1) % N
            W_eff[:, m * C : (m + 1) * C] = \
                (1.0 - pd[m]) * Wm[:, m * C : (m + 1) * C] + \
                pd[pm] * Wm[:, pm * C : (pm + 1) * C]
        feed = x
    else:
        # General path: materialize feat with the true attention weights.
        gap = l1 - l0
        a1 = 1.0 / (1.0 + np.exp(np.clip(-gap, -87.0, 87.0)))
        a0 = 1.0 - a1
        c0 = (a0 * (1.0 - pd))[:, :, None]
        c1 = (a0 * pd + a1 * ld * (1.0 - pd))[:, :, None]
        c2 = (a1 * ld * pd)[:, :, None]
        feed = c0 * x + c1 * x1 + c2 * np.roll(x, -2, axis=1)
        W_eff = Wm

    nsplit = FP8_CHUNKS * C
    feed = feed.reshape(B, NCh, HW)
    # X8: [B, NP8, C, 2*HW], partition row c holds channels (256p+c, 256p+128+c)
    f8part = (feed[:, :nsplit, :] * SX).reshape(B, NP8, 2, C, HW)
    X8 = np.ascontiguousarray(f8part.transpose(0, 1, 3, 2, 4)).reshape(
        B, NP8, C, 2 * HW
    )
    X8 = np.clip(X8, -240, 240).astype(e4)
    X16 = np.ascontiguousarray(
        (feed[:, nsplit:, :] * SX).reshape(B, max(N16, 1), C, HW)
    ).astype(np.float16)

    WT = (W_eff * SW).T  # [c_in, o]
    W8 = WT[:nsplit].reshape(NP8, 2, C, NCh).transpose(0, 2, 1, 3)
    W8 = np.clip(np.ascontiguousarray(W8).reshape(NP8, C, 2 * NCh), -240, 240).astype(e4)
    W16 = np.ascontiguousarray(WT[nsplit:].reshape(max(N16, 1), C, NCh)).astype(
        np.float16
    )
    bias_t = np.ascontiguousarray(
        np.asarray(conv_b, dtype=np.float32).reshape(N, C).T
    )  # [C, N]: column ob = biases of output block ob

    if "nc" not in _cache:
        _cache["nc"] = _build_nc()
    nc = _cache["nc"]

    in_maps = [
        {
            "xs8": X8[c * BB : (c + 1) * BB],
            "xs16": X16[c * BB : (c + 1) * BB],
            "wt8": W8,
            "wt16": W16,
            "bias": bias_t,
        }
        for c in range(NCORES)
    ]
    res = None
    for attempt in range(3):
        try:
            res = run_bass_kernel_spmd(
                nc, in_maps, core_ids=list(range(NCORES)), trace=TRACE,
                trace_cores=TRACE_CORES,
            )
            break
        except Exception:
            # The PJRT/axon dispatch occasionally hits a transient
            # device-unrecoverable error; a retry re-initializes and succeeds.
            if attempt == 2:
                raise
            import time

            time.sleep(2.0)
    LAST_RESULT = res
    slabs = np.concatenate(
        [np.asarray(res.results[c]["out"]) for c in range(NCORES)], axis=0
    )  # [B, N, NT, C, 512]
    out = np.empty((B, NCh, HW), np.float32)
    for t, (c0, cw) in enumerate(COLT):
        out[:, :, c0 : c0 + cw] = (
            slabs[:, :, t, :, :cw].reshape(B, NCh, cw).astype(np.float32)
        )
    out = out + feed if saturated else out + x.reshape(B, NCh, HW)
    # residual is x in both paths; in the saturated path feed IS x.
    return out.reshape(B, NCh, H, W)
